# revision 2
# baseline (speedup 1.0000x reference)
"""CRF NLL loss on 8 Trainium2 NeuronCores — v2 (flat-slot variant).

Per core (64 seqs), ONE dual-direction scan computes BOTH the
log-partition (forward algorithm) and the gold path score:

  partitions 0..63   : forward direction (t = 1 .. 512)
  partitions 64..127 : backward direction (t = 1022 .. 513)
  stream 0 ("state"): 64 columns = full alpha/beta vectors (exp space)
  stream 1 ("gold") : 64 columns = one-hot path columns: the same
      recurrence with emissions MASKED by onehot(target)*OH_SCALE —
      accumulates exp(gold score) exactly.

Round r per stream: ONE [K=128, M=128, N=64] matmul against a resident
block-diagonal stationary diag(exp(Tr), exp(Tr)^T), then ONE DVE mult
[128, 64]: stream 0 with et = exp(emit - C), stream 1 with
ohE = onehot * OH_SCALE * et (flat tiles, no interleaving). The two
directions meet in the middle; the meet dot supplies the middle
transition pair for both streams. Per-seq renorms feed Macc;
ln(dot) + Macc gives per-seq logZ-core and gold-core.

loss = sum_b [logZ_core - gold_core] + B*1022*ln(OH_SCALE)  (host)
"""

import os
import sys

if "/opt/trn_rl_repo" not in sys.path:
    sys.path.insert(0, "/opt/trn_rl_repo")

import numpy as np

B, T, L = 512, 1024, 64
NCORES = 8
BL = B // NCORES            # 64 seqs per core
NR = 512                    # rounds per direction
RC = 64                     # rounds per chunk
NCHUNK = NR // RC
C_SHIFT = 5.2
RENORM_GOLD = 16            # -15-transition bursts underflow bf16 fast
RENORM_STATE = 128
T_START, T_STOP = 1, 2
OH_SCALE = 288.0            # bf16-exact ~e^{C+0.47}: absorbs the -15-row
                            # transition penalties' expected gold decay
DEBUG_TAP = os.environ.get("V2_TAP", "") == "1"

_CACHE = {}


def _split_multi_waits(nc, mybir, max_waits=1):
    """Walrus encodes one sync-wait per instruction: elide waits satisfied
    by same-engine program order, hoist the rest onto NoOps."""
    n_split = 0
    for f in nc.m.functions:
        for bb in f.blocks:
            insts = list(bb.instructions)
            inc_count = {}
            out = []
            changed = False
            for ins in insts:
                si = getattr(ins, "sync_info", None)
                waits = list(si.on_wait) if si is not None and si.on_wait else []
                if waits and str(ins.engine) != "EngineType.PE":
                    eng = str(ins.engine)
                    kept = []
                    for w in waits:
                        key = (eng, w.ant_name)
                        if (
                            w.wait_mode == "sem-ge-imm"
                            and inc_count.get(key, 0) >= (w.wait_value or 0)
                        ):
                            changed = True
                            continue
                        kept.append(w)
                    waits = kept
                    if len(waits) != len(si.on_wait):
                        si.on_wait = waits
                if len(waits) > max_waits:
                    keep = waits[len(waits) - max_waits:]
                    hoist = waits[: len(waits) - max_waits]
                    for i, w in enumerate(hoist):
                        nop = mybir.InstNoOp(name=f"{ins.name}-hw{i}", ins=[], outs=[])
                        nop.engine = ins.engine
                        nop.sync_info = mybir.SyncInfo(on_wait=[w], on_update=[])
                        out.append(nop)
                    si.on_wait = keep
                    changed = True
                    n_split += 1
                out.append(ins)
                if si is not None and si.on_update:
                    eng = str(ins.engine)
                    for u in si.on_update:
                        if getattr(u, "update_mode", None) == "sem-inc":
                            key = (eng, u.ant_name)
                            inc_count[key] = inc_count.get(key, 0) + (
                                u.update_value or 0
                            )
            if changed:
                bb.instructions = out
    return n_split


def _build():
    import concourse.bass as bass
    import concourse.mybir as mybir
    import concourse.tile as tile

    fp32 = mybir.dt.float32
    bf16 = mybir.dt.bfloat16
    AOP = mybir.AluOpType
    AF = mybir.ActivationFunctionType

    nc = bass.Bass()
    emitfb_d = nc.dram_tensor("emitfb", [128, NR * BL], fp32, kind="ExternalInput")
    tgtfb_d = nc.dram_tensor(
        "tgtfb", [2 * NCHUNK, RC * BL], bf16, kind="ExternalInput"
    )
    trans_d = nc.dram_tensor("transition", [L, L], fp32, kind="ExternalInput")
    tagcol_d = nc.dram_tensor("tagcol", [128, 1], fp32, kind="ExternalInput")
    ident_d = nc.dram_tensor("ident2", [128, 64], bf16, kind="ExternalInput")
    hsel_d = nc.dram_tensor("hsel", [128, 2], bf16, kind="ExternalInput")
    hselT_d = nc.dram_tensor("hselT", [2, 128], fp32, kind="ExternalInput")
    res_d = nc.dram_tensor("res_row", [1, 2 * BL], fp32, kind="ExternalOutput")
    if DEBUG_TAP:
        dbg_d = nc.dram_tensor("dbg", [128, 192], fp32, kind="ExternalOutput")

    noldw = []  # mm stream, None = weight clobber marker

    with tile.TileContext(nc) as tc:
        with (
            tc.tile_pool(name="constp", bufs=1) as constp,
            tc.tile_pool(name="efp", bufs=2) as efp,
            tc.tile_pool(name="etp", bufs=2) as etp,
            tc.tile_pool(name="ohp", bufs=2) as ohp,
            tc.tile_pool(name="ohep", bufs=2) as ohep,
            tc.tile_pool(name="repp", bufs=2) as repp,
            tc.tile_pool(name="ppa", bufs=3) as ppa,
            tc.tile_pool(name="ppb", bufs=3) as ppb,
            tc.tile_pool(name="smallp", bufs=4) as smallp,
            tc.tile_pool(name="ps_u0", bufs=3, space="PSUM") as ps_u0,
            tc.tile_pool(name="ps_s", bufs=1, space="PSUM") as ps_s,
            tc.tile_pool(name="ps_r", bufs=1, space="PSUM") as ps_r,
        ):
            # ---- constants -------------------------------------------------
            T2_sb = constp.tile([128, L], fp32)
            nc.sync.dma_start(T2_sb[0:L, :], trans_d[:])
            nc.sync.dma_start(T2_sb[L:, :], trans_d[:].rearrange("a b -> b a"))
            Wblk = constp.tile([128, 128], bf16)
            nc.vector.memset(Wblk[:], 0.0)
            nc.scalar.activation(Wblk[0:L, 0:L], T2_sb[0:L, :], AF.Exp)
            nc.scalar.activation(Wblk[L:, L:], T2_sb[L:, :], AF.Exp)
            initcol = constp.tile([128, 1], fp32)
            nc.sync.dma_start(
                initcol[0:L, :],
                trans_d[T_START: T_START + 1, :].rearrange("a b -> b a"),
            )
            nc.sync.dma_start(initcol[L:, :], trans_d[:, T_STOP: T_STOP + 1])
            einit = constp.tile([128, 1], fp32)
            nc.scalar.activation(einit[:], initcol[:], AF.Exp)
            ones2 = constp.tile([128, 1], bf16)
            nc.vector.memset(ones2[:], 1.0)
            negC = constp.tile([128, 1], fp32)
            nc.vector.memset(negC[:], -C_SHIFT)
            tagf = constp.tile([128, 1], fp32)
            nc.sync.dma_start(tagf[:], tagcol_d[:])
            tagb = constp.tile([128, 1], bf16)
            nc.vector.tensor_copy(tagb[:], tagf[:])
            identS = constp.tile([128, 64], bf16)
            nc.sync.dma_start(identS[:], ident_d[:])
            hsel = constp.tile([128, 2], bf16)
            nc.sync.dma_start(hsel[:], hsel_d[:])
            hselT = constp.tile([2, 128], fp32)
            nc.sync.dma_start(hselT[:], hselT_d[:])
            MaccA = constp.tile([2, BL], fp32)
            nc.vector.memset(MaccA[:], 0.0)
            MaccB = constp.tile([2, BL], fp32)
            nc.vector.memset(MaccB[:], 0.0)
            Macc = {0: MaccA, 1: MaccB}

            p_cur = {0: None, 1: None}
            u_final = {0: None, 1: None}
            pools = {0: ppa, 1: ppb}

            for ci in range(NCHUNK):
                r0 = ci * RC
                ef = efp.tile([128, RC * BL], fp32, name="ef")
                nc.gpsimd.dma_start(ef[:], emitfb_d[:, r0 * BL: (r0 + RC) * BL])
                et = etp.tile([128, RC * BL], bf16, name="et")
                nc.scalar.activation(et[:], ef[:], AF.Exp, bias=negC[:])
                rep = repp.tile([128, RC * BL], bf16, name="rep")
                nc.gpsimd.dma_start(
                    rep[0:64, :],
                    tgtfb_d[2 * ci: 2 * ci + 1, :].to_broadcast([64, RC * BL]),
                )
                nc.gpsimd.dma_start(
                    rep[64:128, :],
                    tgtfb_d[2 * ci + 1: 2 * ci + 2, :].to_broadcast(
                        [64, RC * BL]
                    ),
                )
                oh = ohp.tile([128, RC * BL], bf16, name="oh")
                nc.vector.tensor_tensor(
                    oh[:], rep[:], tagb[:].to_broadcast([128, RC * BL]),
                    AOP.is_equal,
                )
                nc.vector.tensor_scalar(oh[:], oh[:], OH_SCALE, None, AOP.mult)
                ohE = ohep.tile([128, RC * BL], bf16, name="ohE")
                nc.vector.tensor_tensor(ohE[:], oh[:], et[:], AOP.mult)
                src = {0: et, 1: ohE}
                if DEBUG_TAP and ci == 0:
                    dtap = constp.tile([128, 192], fp32)

                rlist = range(1, RC) if ci == 0 else range(r0, r0 + RC)
                if ci == 0:
                    for s in (0, 1):
                        p1 = pools[s].tile([128, BL], bf16, name="p")
                        nc.vector.tensor_scalar(
                            p1[:], src[s][:, 0:BL], einit[:], None, AOP.mult
                        )
                        p_cur[s] = p1

                for r in rlist:
                    rr = r - r0
                    for s in (0, 1):
                        if r == NR - 1:
                            u_final[s] = p_cur[s]
                        U = ps_u0.tile([128, BL], fp32, name="U")
                        mm = nc.tensor.matmul(
                            U[:],
                            lhsT=Wblk[:],
                            rhs=p_cur[s][:],
                            start=True,
                            stop=True,
                            skip_group_check=True,
                        )
                        noldw.append(mm)
                        p_new = pools[s].tile([128, BL], bf16, name="p")
                        msl = src[s][:, rr * BL: (rr + 1) * BL]
                        if r == NR - 2 and s == 1:
                            # gold: upper mask; lower = raw mm (bwd meet)
                            nc.vector.tensor_tensor(
                                p_new[0:64, :], U[0:64, :],
                                src[s][0:64, rr * BL: (rr + 1) * BL], AOP.mult,
                            )
                            nc.vector.tensor_copy(
                                p_new[64:128, :], U[64:128, :]
                            )
                        elif r == NR - 1:
                            nc.vector.tensor_tensor(
                                p_new[0:64, :], U[0:64, :],
                                src[s][0:64, rr * BL: (rr + 1) * BL], AOP.mult,
                            )
                        else:
                            nc.vector.tensor_tensor(
                                p_new[:], U[:], msl, AOP.mult
                            )
                        p_cur[s] = p_new
                        if DEBUG_TAP and s == 1 and r in (32, 96, 130):
                            col = {32: 0, 96: 64, 130: 128}[r]
                            nc.vector.tensor_copy(
                                dtap[:, col: col + 64], p_new[:]
                            )
                            if r == 130:
                                nc.sync.dma_start(dbg_d[:], dtap[:])

                    rn = []
                    if (r + 1) % RENORM_GOLD == 0 and r < NR - 2:
                        rn.append(1)
                    if (r + 1) % RENORM_STATE == 0 and r < NR - 2:
                        rn.append(0)
                    if rn:
                        for s in rn:
                            S = ps_s.tile([2, BL], fp32, name="S")
                            nc.tensor.matmul(
                                S[:], lhsT=hsel[:], rhs=p_cur[s][:],
                                start=True, stop=True, skip_group_check=True,
                            )
                            lnS = smallp.tile([2, BL], fp32, name="lnS")
                            nc.scalar.activation(lnS[:], S[:], AF.Ln)
                            nc.vector.tensor_tensor(
                                Macc[s][:], Macc[s][:], lnS[:], AOP.add
                            )
                            rS = smallp.tile([2, BL], fp32, name="rS")
                            nc.vector.reciprocal(rS[:], S[:])
                            R2 = ps_r.tile([128, BL], fp32, name="R2")
                            nc.tensor.matmul(
                                R2[:], lhsT=hselT[:], rhs=rS[:],
                                start=True, stop=True, skip_group_check=True,
                            )
                            p_rn = pools[s].tile([128, BL], bf16, name="p")
                            nc.vector.tensor_tensor(
                                p_rn[:], p_cur[s][:], R2[:], AOP.mult
                            )
                            p_cur[s] = p_rn
                        noldw.append(None)

            # ---- end game --------------------------------------------------
            # move bwd-final halves to partitions 0..63 via identity matmul
            U2 = ps_r.tile([64, 128], fp32, name="U2")
            nc.tensor.matmul(
                U2[:, 0:64], lhsT=identS[:], rhs=u_final[0][:],
                start=True, stop=True, skip_group_check=True,
            )
            nc.tensor.matmul(
                U2[:, 64:128], lhsT=identS[:], rhs=u_final[1][:],
                start=True, stop=True, skip_group_check=True,
            )
            prod = constp.tile([64, 128], bf16)
            nc.vector.tensor_tensor(
                prod[:, 0:64], p_cur[0][0:64, :], U2[:, 0:64], AOP.mult
            )
            nc.vector.tensor_tensor(
                prod[:, 64:128], p_cur[1][0:64, :], U2[:, 64:128], AOP.mult
            )
            D = ps_s.tile([1, 128], fp32, name="D")
            nc.tensor.matmul(
                D[:], lhsT=ones2[0:64, :], rhs=prod[:],
                start=True, stop=True, skip_group_check=True,
            )
            lnD = smallp.tile([1, 128], fp32)
            nc.scalar.activation(lnD[:], D[:], AF.Ln)
            ones2f = constp.tile([2, 1], fp32)
            nc.vector.memset(ones2f[:], 1.0)
            Msum = ps_r.tile([1, 128], fp32, name="Msum")
            nc.tensor.matmul(
                Msum[:, 0:64], lhsT=ones2f[:], rhs=MaccA[:],
                start=True, stop=True, skip_group_check=True,
            )
            nc.tensor.matmul(
                Msum[:, 64:128], lhsT=ones2f[:], rhs=MaccB[:],
                start=True, stop=True, skip_group_check=True,
            )
            res = constp.tile([1, 128], fp32)
            nc.vector.tensor_tensor(res[:], lnD[:], Msum[:], AOP.add)
            nc.sync.dma_start(res_d[:], res[:])

    # ldweights=False on back-to-back same-weight matmuls (inert on this
    # walrus, but harmless)
    first = True
    reload_next = False
    for mm in noldw:
        if mm is None:
            reload_next = True
            continue
        if first or reload_next:
            first = False
            reload_next = False
            continue
        try:
            (mm.ins if hasattr(mm, "ins") else mm).ldweights = False
        except Exception:
            break

    _split_multi_waits(nc, mybir)
    return nc


def _get_nc():
    if "nc" not in _CACHE:
        _CACHE["nc"] = _build()
    return _CACHE["nc"]


def kernel(emit, target, transition):
    import ml_dtypes

    from concourse import bass_utils

    emit = np.ascontiguousarray(emit, dtype=np.float32)
    tgt = np.ascontiguousarray(target).astype(np.int32)
    trans = np.ascontiguousarray(transition, dtype=np.float32)
    assert emit.shape == (B, T, L) and tgt.shape == (B, T)

    nc = _get_nc()
    in_maps = []
    for k in range(NCORES):
        esh = emit[k * BL: (k + 1) * BL]          # [64, 1024, 64]
        tsh = tgt[k * BL: (k + 1) * BL]           # [64, 1024]
        eT = esh.transpose(2, 1, 0)               # [j, t, b]
        efb = np.empty((128, NR, BL), np.float32)
        efb[:64] = eT[:, 1: NR + 1, :]
        efb[64:, :510] = eT[:, 1022:512:-1, :]
        efb[64:, 510] = C_SHIFT
        efb[64:, 511] = 0.0
        tfb = np.empty((2, NR, BL), np.float32)
        tfb[0] = tsh[:, 1: NR + 1].T
        tfb[1, :510] = tsh[:, 1022:512:-1].T
        tfb[1, 510:] = -1.0
        in_maps.append(
            {
                "emitfb": np.ascontiguousarray(efb.reshape(128, NR * BL)),
                "tgtfb": np.ascontiguousarray(
                    tfb.reshape(2, NCHUNK, RC * BL)
                    .transpose(1, 0, 2)
                    .reshape(2 * NCHUNK, RC * BL)
                ).astype(ml_dtypes.bfloat16),
                "transition": trans,
                "tagcol": (np.arange(128, dtype=np.float32) % 64).reshape(128, 1),
                "ident2": np.concatenate(
                    [np.zeros((64, 64), np.float32), np.eye(64, dtype=np.float32)]
                ).astype(ml_dtypes.bfloat16),
                "hsel": np.stack(
                    [(np.arange(128) < 64).astype(np.float32),
                     (np.arange(128) >= 64).astype(np.float32)], 1
                ).astype(ml_dtypes.bfloat16),
                "hselT": np.stack(
                    [(np.arange(128) < 64).astype(np.float32),
                     (np.arange(128) >= 64).astype(np.float32)]
                ),
            }
        )
    res = bass_utils.run_bass_kernel_spmd(nc, in_maps, core_ids=list(range(NCORES)))

    if DEBUG_TAP:
        r0 = res.results[0]
        d = r0["dbg"]
        print("TAP rep[0:4,0:6]:", d[0:4, 0:6])
        print("TAP rep[64:68,0:6]:", d[64:68, 0:6])
        print("TAP oh nonzero count:", (d[:, 64:128] != 0).sum(), "/ 8192")
        print("TAP ohE nonzero count:", (d[:, 128:192] != 0).sum())

    tot = 0.0
    for r in res.results:
        row = r["res_row"].astype(np.float64).reshape(2, BL)
        if os.environ.get("V2_DEBUG"):
            print("state[:6]:", np.round(row[0, :6], 2),
                  "gold[:6]:", np.round(row[1, :6], 2))
        tot += float(row[0].sum()) - float(row[1].sum())
    tot += B * 1022.0 * float(np.log(OH_SCALE))
    return np.float32(tot)
